# revision 11
# baseline (speedup 1.0000x reference)
"""CQAttention (QANet context-query attention) Trainium2 kernel, v6.1.

Full-input contract: kernel(**inputs) takes the unsharded arrays
  C [64, 1024, 256] f32, Q [64, 128, 256] f32,
  cmask [64, 1024] f32 (unused by the reference), qmask [64, 128] f32,
  w [768] f32
and returns out [64, 1024, 512] f32.

Sharding: batch dim across 8 NeuronCores (8 batches/core), no
cross-core communication.

Design (history: v5 f32-I/O 91us, v6.0 73.6us):
  - All big DRAM I/O in bf16 (rel-err gate 2e-2; bf16 adds ~0.4%).
  - Everything computed transposed: host supplies C^T and Q^T packed so
    the kernel never transposes on-chip; output leaves as [2D, LC]^T and
    the host transposes back (layout only, no host FLOPs).
  - DMA queues are packet-rate-limited (~170ns/packet/DGE-engine
    measured), so host packs arrays to give 4KB (loads) / 8KB (store)
    contiguous runs per partition, and the whole batch output goes out
    in ONE dma_start.
  - Per batch: S^T = (w3*Q)^T @ C^T (4 mm) -> exp+bias (2 ACT) ->
    d = ones@E replicated across partitions (2 mm) ->
    r = reciprocal_approx_fast (1 DVE) -> En = E*r (DVE+POOL) ->
    A^T = Q @ En (4 mm, normalized) -> evac copies (4 ACT) +
    CA = u*C^T from PSUM (3 DVE) / from SBUF (1 POOL).
  - A-phase lags one iteration so the PE never waits on recip/Enorm.
    PE order per iter: d(b), S(b+1), A(b-1).
"""

from contextlib import ExitStack

import numpy as np
import ml_dtypes

import concourse.bacc as bacc
import concourse.bass as bass
import concourse.mybir as mybir
import concourse.tile as tile
from concourse.bass_utils import run_bass_kernel_spmd
from concourse.masks import make_identity

B, LC, LQ, D = 64, 1024, 128, 256
N_CORES = 8
BL = B // N_CORES  # batches per core
KD = D // 128      # d-chunks
F32 = mybir.dt.float32
BF16 = mybir.dt.bfloat16
MULT = mybir.AluOpType.mult
ADD = mybir.AluOpType.add
EXP = mybir.ActivationFunctionType.Exp
BF = ml_dtypes.bfloat16

_CACHE: dict = {}


def _build_bass() -> bass.Bass:
    nc = bacc.Bacc("TRN2")
    # CTC[b, p, k, i] = C[b, i, 128k+p] * nothing (raw C^T, packed so each
    # partition reads 4KB contiguous).
    CT_h = nc.dram_tensor("CTC", [BL, 128, KD, LC], BF16, kind="ExternalInput")
    Q_h = nc.dram_tensor("Qb", [LQ, BL, D], BF16, kind="ExternalInput")
    QT_h = nc.dram_tensor("QT", [128, KD, BL, LQ], BF16, kind="ExternalInput")
    qm_h = nc.dram_tensor("qmask", [BL, LQ], F32, kind="ExternalInput")
    w_h = nc.dram_tensor("w", [3 * D], F32, kind="ExternalInput")
    # OTP[b, p, k, c, i]: c in {A, CA} per d-chunk k; per-k stores are
    # 4KB/partition contiguous runs.
    out_h = nc.dram_tensor("outT", [BL, 128, KD, 2, LC], BF16, kind="ExternalOutput")

    with tile.TileContext(nc) as tc, ExitStack() as ctx:
        singles = ctx.enter_context(tc.tile_pool(name="singles", bufs=1))
        ct_pool = ctx.enter_context(tc.tile_pool(name="ct", bufs=5))
        e_pool = ctx.enter_context(tc.tile_pool(name="e", bufs=3))
        en_pool = ctx.enter_context(tc.tile_pool(name="en", bufs=2))
        r_pool = ctx.enter_context(tc.tile_pool(name="r", bufs=2))
        o_pool = ctx.enter_context(tc.tile_pool(name="o", bufs=3))
        small_pool = ctx.enter_context(tc.tile_pool(name="small", bufs=12))
        scratch_pool = ctx.enter_context(tc.tile_pool(name="scr", bufs=2))
        s_pool = ctx.enter_context(tc.tile_pool(name="s", bufs=2, space="PSUM"))
        u_pool = ctx.enter_context(tc.tile_pool(name="u", bufs=2, space="PSUM"))

        # ---------------- one-time setup ----------------
        ident32 = singles.tile([128, 128], F32)
        make_identity(nc, ident32)
        allones = singles.tile([128, 128], BF16)
        nc.vector.memset(allones, 1.0)
        one1 = singles.tile([1, 1], F32)
        nc.vector.memset(one1, 1.0)
        ones_row = singles.tile([1, 128], F32)
        nc.vector.memset(ones_row, 1.0)

        w_row = singles.tile([1, 3 * D], F32)
        nc.sync.dma_start(
            out=w_row, in_=bass.AP(tensor=w_h, offset=0, ap=[[1, 1], [1, 3 * D]])
        )
        qm8 = singles.tile([BL, LQ], F32)
        nc.sync.dma_start(
            out=qm8, in_=bass.AP(tensor=qm_h, offset=0, ap=[[LQ, BL], [1, LQ]])
        )
        qt_all = singles.tile([128, KD, BL, LQ], BF16)
        nc.sync.dma_start(
            out=qt_all,
            in_=bass.AP(
                tensor=QT_h,
                offset=0,
                ap=[[KD * BL * LQ, 128], [BL * LQ, KD], [1, BL * LQ]],
            ),
        )
        q_all = singles.tile([128, BL, D], BF16)
        nc.sync.dma_start(
            out=q_all,
            in_=bass.AP(
                tensor=Q_h, offset=0, ap=[[BL * D, 128], [D, BL], [1, D]]
            ),
        )

        c_tiles = [None] * BL

        def load_ct(b):
            ct_t = ct_pool.tile([128, KD, LC], BF16, name="ct")
            nc.sync.dma_start(out=ct_t, in_=CT_h[b])
            c_tiles[b] = ct_t

        load_ct(0)
        load_ct(1)
        load_ct(2)

        # w3T[p, k] = w[2D + 128k + p]; w2rep[p, :] = w2 broadcast.
        wps = s_pool.tile([128, KD + D], F32, name="wps", tag="s")
        for k in range(KD):
            nc.tensor.matmul(
                wps[:, k : k + 1],
                w_row[:, 2 * D + 128 * k : 2 * D + 128 * (k + 1)],
                one1,
                start=True,
                stop=True,
            )
        nc.tensor.matmul(
            wps[:, KD:], ones_row, w_row[:, D : 2 * D], start=True, stop=True
        )
        w3T = singles.tile([128, KD], F32)
        nc.vector.tensor_copy(out=w3T, in_=wps[:, :KD])
        w2rep = singles.tile([128, D], F32)
        nc.vector.tensor_copy(out=w2rep, in_=wps[:, KD:])

        qmT_ps = u_pool.tile([128, BL], F32, name="qmT_ps", tag="u")
        nc.tensor.matmul(qmT_ps, qm8, ident32[0:BL, 0:BL], start=True, stop=True)
        qmT = singles.tile([128, BL], F32)
        nc.vector.tensor_copy(out=qmT, in_=qmT_ps)

        # qtw3[p, k, b, j] = w3[128k+p] * Q^T[128k+p, b, j].
        # Batch 0 first (tiny, unblocks S(0)); batches 1.. in one bulk op
        # per k while the head of the pipeline is otherwise idle on DVE.
        qtw3 = singles.tile([128, KD, BL, LQ], BF16)

        def qtw3_prep(b_slice, eng):
            for k in range(KD):
                eng.tensor_scalar_mul(
                    out=qtw3[:, k, b_slice],
                    in0=qt_all[:, k, b_slice],
                    scalar1=w3T[:, k : k + 1],
                )

        bias_all = singles.tile([128, BL], F32)

        def qprep(b):
            q2sb = small_pool.tile([128, 1], F32, name="q2sb")
            scr = scratch_pool.tile([128, D], F32, name="scr")
            nc.vector.scalar_tensor_tensor(
                out=scr,
                in0=q_all[:, b],
                scalar=1.0,
                in1=w2rep,
                op0=MULT,
                op1=MULT,
                accum_out=q2sb,
            )
            nc.vector.scalar_tensor_tensor(
                out=bias_all[:, b : b + 1],
                in0=qmT[:, b : b + 1],
                scalar=-10000.0,
                in1=q2sb,
                op0=MULT,
                op1=ADD,
            )

        # ---------------- per-batch pipeline stages ----------------
        def stage_s(b):
            ct_t = c_tiles[b]
            s_t = s_pool.tile([128, 2, 512], F32, name="s_t", tag="s")
            for h in range(2):
                for k in range(KD):
                    nc.tensor.matmul(
                        s_t[:, h],
                        qtw3[:, k, b],
                        ct_t[:, k, 512 * h : 512 * (h + 1)],
                        start=(k == 0),
                        stop=(k == KD - 1),
                    )
            e_t = e_pool.tile([128, 2, 512], BF16)
            nc.scalar.activation(
                out=e_t, in_=s_t, func=EXP,
                bias=bias_all[:, b : b + 1], scale=1.0,
            )
            return e_t

        def stage_d_mm(b, e_t):
            d_ps = s_pool.tile([128, 2, 512], F32, name="d_ps", tag="s")
            for h in range(2):
                nc.tensor.matmul(
                    d_ps[:, h], allones, e_t[:, h], start=True, stop=True
                )
            return d_ps

        def stage_d_vec(b, e_t, d_ps):
            """Per-half reciprocal; half 1 of E is pre-normalized on POOL
            (half 0 gets its 1/d folded into the A-evacuation on DVE)."""
            r_t = r_pool.tile([128, 2, 512], F32)
            for h in range(2):
                nc.vector.reciprocal_approx_fast(out=r_t[:, h], in_=d_ps[:, h])
            en_t = en_pool.tile([128, 2, 512], BF16)
            nc.vector.tensor_tensor(
                out=en_t[:, 0], in0=e_t[:, 0], in1=r_t[:, 0], op=MULT
            )
            nc.gpsimd.tensor_tensor(
                out=en_t[:, 1], in0=e_t[:, 1], in1=r_t[:, 1], op=MULT
            )
            return en_t

        def stage_a(b, en_t):
            """A^T matmuls on pre-normalized E; 4 plain ACT copy evacs;
            CA: half 0 on POOL, half 1 on DVE (both SBUF bf16)."""
            ct_t = c_tiles[b]
            o_t = o_pool.tile([128, KD, 2, LC], BF16)
            for k in range(KD):
                u_t = u_pool.tile([128, 2, 512], F32, name="u_t", tag="u")
                lhs = q_all[:, b, 128 * k : 128 * (k + 1)]
                for h in range(2):
                    nc.tensor.matmul(
                        u_t[:, h], lhs, en_t[:, h], start=True, stop=True
                    )
                sl0 = slice(0, 512)
                sl1 = slice(512, 1024)
                nc.scalar.copy(out=o_t[:, k, 0, sl0], in_=u_t[:, 0])
                nc.scalar.copy(out=o_t[:, k, 0, sl1], in_=u_t[:, 1])
                nc.gpsimd.tensor_tensor(
                    out=o_t[:, k, 1, sl0], in0=o_t[:, k, 0, sl0],
                    in1=ct_t[:, k, sl0], op=MULT,
                )
                nc.vector.tensor_tensor(
                    out=o_t[:, k, 1, sl1], in0=o_t[:, k, 0, sl1],
                    in1=ct_t[:, k, sl1], op=MULT,
                )
            return o_t

        def store_o(b, o_t):
            for k in range(KD):
                nc.sync.dma_start(out=out_h[b, :, k], in_=o_t[:, k])

        # ---------------- software-pipelined emission ----------------
        qprep(0)
        qtw3_prep(slice(0, 1), nc.vector)
        qprep(1)
        e_tiles = {0: stage_s(0)}
        qtw3_prep(slice(1, BL), nc.vector)  # bulk, off the critical path
        d_cur = stage_d_mm(0, e_tiles[0])
        en_tiles = {0: stage_d_vec(0, e_tiles[0], d_cur)}
        pending_store = None
        for b in range(BL + 1):
            if b + 3 < BL:
                load_ct(b + 3)
            if pending_store is not None:
                store_o(*pending_store)
                pending_store = None
            # PE order: d(b) first (inputs long ready), then S(b+1), then
            # A(b-1); recip/Enorm(b) start right after d(b).
            if 1 <= b < BL:
                d_ps = stage_d_mm(b, e_tiles[b])
                r_en = (b, e_tiles.pop(b), d_ps)
            else:
                r_en = None
            if b + 1 < BL:
                e_tiles[b + 1] = stage_s(b + 1)
            if r_en is not None:
                en_tiles[b] = stage_d_vec(*r_en)
            if b >= 1:
                pending_store = (b - 1, stage_a(b - 1, en_tiles.pop(b - 1)))
            if b + 2 < BL:
                qprep(b + 2)
        store_o(*pending_store)
    nc.compile()
    return nc


def _get_bass() -> bass.Bass:
    if "nc" not in _CACHE:
        _CACHE["nc"] = _build_bass()
    return _CACHE["nc"]


def _prep_core_inputs(C, Q, qmask, w, c):
    sl = slice(c * BL, (c + 1) * BL)
    Cc = C[sl]                                   # [BL, LC, D] f32
    # CTC[b, p, k, i] = C[b, i, 128k+p]
    CT = Cc.transpose(0, 2, 1).astype(BF)        # [BL, D, LC]
    CTC = np.ascontiguousarray(
        CT.reshape(BL, KD, 128, LC).transpose(0, 2, 1, 3)
    )                                            # [BL, 128, KD, LC]
    Qc = Q[sl].astype(BF)                        # [BL, LQ, D] bf16
    Qp = np.ascontiguousarray(Qc.transpose(1, 0, 2))   # [LQ, BL, D]
    QT = np.ascontiguousarray(
        Qc.transpose(2, 0, 1).reshape(KD, 128, BL, LQ).transpose(1, 0, 2, 3)
    )                                                  # [128, KD, BL, LQ]
    return {
        "CTC": CTC,
        "Qb": Qp,
        "QT": QT,
        "qmask": np.ascontiguousarray(qmask[sl], dtype=np.float32),
        "w": np.ascontiguousarray(w, dtype=np.float32),
    }


def _run(C, Q, qmask, w, trace=False, **spmd_kwargs):
    nc = _get_bass()
    C = np.ascontiguousarray(C, dtype=np.float32)
    Q = np.ascontiguousarray(Q, dtype=np.float32)
    qmask = np.ascontiguousarray(qmask, dtype=np.float32)
    w = np.ascontiguousarray(w, dtype=np.float32)
    in_maps = [_prep_core_inputs(C, Q, qmask, w, c) for c in range(N_CORES)]
    res = run_bass_kernel_spmd(
        nc, in_maps, list(range(N_CORES)), trace=trace, **spmd_kwargs
    )
    outT = np.concatenate(
        [np.asarray(res.results[c]["outT"]) for c in range(N_CORES)], axis=0
    )  # [B, 128, KD, 2, LC] bf16: [b, p, k, (a|ca), i]
    # out[b, i, 256*c + 128*k + p] = outT[b, p, k, c, i]
    out = np.ascontiguousarray(
        outT.astype(np.float32).transpose(0, 4, 3, 2, 1).reshape(B, LC, 2 * D)
    )
    return out, res


def kernel(C, Q, cmask, qmask, w):
    out, _ = _run(C, Q, qmask, w, trace=False)
    return out
